# revision 10
# baseline (speedup 1.0000x reference)
"""NearbyAttention on 8 trn2 NeuronCores — v3 (banded, bin-packed).

Sharding: 2 heads per core (16 heads / 8 cores). Each core computes its
2 heads' QKV projections, block-sparse masked attention, and a partial
output projection fused across its 2 heads; the host sums the 8
partials and adds the bias.

Design (vs the v2 baseline, 147us -> ~121us):
- Score/exp/mask/PV work is clipped to the EXACT query band of each
  (q512-chunk, k128-tile) block (14318 of 21504 cols, -33%).
- Per k-tile, both heads' scores go to one PSUM tile [128, 2h, 512];
  the two K=64 score matmuls row-tile concurrently on the PE (the
  second costs ~3ns). One exp instruction covers both heads.
- Adjacent tiles are greedily bin-packed into <=512-col score PSUM
  bins (42 tiles -> 34 bins): fewer exp instructions and fewer
  score->exp serialization boundaries.
- Score PSUM is double-buffered (pool bufs=2, 4 banks) so scores(t+1)
  overlap exp(t); outproj PSUM shares the proj pool (mmps) to free
  those banks. PSUM: mmps 2x1 + sps 2x2 + accps 2x1 banks = 8.
- PV accumulates into accs[65, 512] (64 dh rows + ones-row rowsum) as
  pure accumulates over a PE-zeroed tile (per-region start flags would
  clobber sibling chains sharing the PSUM bank). The zero-openers are
  emitted lazily (after bin 1) so the in-order PE queue is not blocked
  by their WAR on the previous chunk's finalize.
- PV is deferred 3 bins behind scores (pend queue) to absorb the
  exp->mask-mul latency; mask multiplies run on DVE in 2x mode
  (bf16 0/1 tiles, contiguous per-head slices).
- Finalize per half-chunk: rowsum copy (Act) -> reciprocal_approx_fast
  (DVE, from SBUF) -> partition_broadcast (GpSimd) -> normT mul (DVE).
  The last chunk finalizes its first half early and emits its outproj
  immediately, shrinking the tail.
- Outproj: per (128-row, 512-col) matmul, copies alternate Act/DVE,
  one merged [128, 1024] store per row-tile on the sync ring (2KB
  descriptor rows); input loads on the sync ring in d-halves (8KB
  per-partition descriptors via the [128, 5, 8, 512] DRAM layout);
  weights/masks on the gpsimd ring, ordered so q0/k0 land first.
- PE warmup matmuls run on a memset tile (no DMA dependency) so the
  HAM clock-gate opens while the first input chunk is still loading.

Numerics (bf16 everywhere, fp32 PSUM): softmax without
max-subtraction; masked entries killed by multiplying exp(S) with a
0/1 mask tile; query row 2560 / key col 2560 handled on host.
rel err vs fp32 reference: 4.07e-3 (tolerance 2e-2).
"""

import numpy as np
import sys

sys.path.insert(0, "/opt/trn_rl_repo")

import ml_dtypes
import concourse.bass as bass
import concourse.bacc as bacc
import concourse.tile as tile
import concourse.mybir as mybir
from concourse import masks
from concourse.bass_utils import run_bass_kernel_spmd

N_CORES = 8
HEADS = 16
DH = 64
DIM = 1024
HPC = HEADS // N_CORES          # heads per core = 2
E = HPC * DH                    # per-core inner dim = 128
N_FULL = 2561
N = 2560                        # device seq len (row/col 2560 host-handled)
IC = 512                        # query chunk
JT = 128                        # key tile (partition dim)
SUB = 128                       # q sub-block granularity
ND = DIM // 128                 # 8 contraction chunks for projections
N_ICS = N // IC                 # 5
N_JTS = N // JT                 # 20
NSUB = IC // SUB                # 4
SCALE = DH ** -0.5

F32 = mybir.dt.float32
BF = mybir.dt.bfloat16
F8 = mybir.dt.float8e4
BF_NP = ml_dtypes.bfloat16
F8_NP = ml_dtypes.float8_e4m3


def _schedule(mask):
    """Per q512-chunk: list of (jt, lo, hi, u) blocks (lo/hi in q128 sub
    units, u = mask tile index or -1 if the band is fully unmasked), the
    unique 0/1 mask tiles [128k, 512q], and per-(chunk, block) PV
    segments [(a, b, start, stop)] with per-sub accumulation flags."""
    B = ~mask[:N, :N]  # True = attend
    uniq, tiles = {}, []
    chunks = []
    for ic in range(N_ICS):
        lst = []
        for jt in range(N_JTS):
            blk = B[ic * IC:(ic + 1) * IC, jt * JT:(jt + 1) * JT]  # [512q,128k]
            if not blk.any():
                continue
            rows = np.where(blk.any(axis=1))[0]
            lo, hi = int(rows[0]), int(rows[-1]) + 1  # exact columns
            if blk[lo:hi].all():
                u = -1
            else:
                key = blk.tobytes()
                if key not in uniq:
                    uniq[key] = len(uniq)
                    tiles.append(blk.T.astype(np.float32))  # [128k, 512q]
                u = uniq[key]
            lst.append((jt, lo, hi, u))
        chunks.append(lst)
    mb = (np.stack(tiles).transpose(1, 0, 2) if tiles
          else np.zeros((JT, 1, IC), np.float32))  # [128k, n_u, 512q]
    mb = np.ascontiguousarray(mb)

    binned = []
    for ic, lst in enumerate(chunks):
        covered = np.zeros(IC, bool)
        for (jt, lo, hi, u) in lst:
            covered[lo:hi] = True
        assert covered.all(), f"chunk {ic} has uncovered queries"
        # greedy bin-pack adjacent tiles into <=512-col score psum bins,
        # preserving jt order (PV accumulation order must follow jt)
        bins = []
        cur, cw = [], 0
        for (jt, lo, hi, u) in lst:
            w = hi - lo
            if cur and cw + w > IC:
                bins.append(cur)
                cur, cw = [], 0
            cur.append((jt, lo, hi, u, cw))
            cw += w
        if cur:
            bins.append(cur)
        # last bin index whose tiles touch the first half-chunk; used to
        # place the early fin_half(0) emission
        bclose = max(t for t, bn in enumerate(bins)
                     if any(lo < IC // 2 for (jt, lo, hi, u, off) in bn))
        binned.append((bins, bclose))
    return binned, mb, None


def _build(chunks, pv_segs, n_mb):
    nc = bacc.Bacc("TRN2", target_bir_lowering=False, debug=False,
                   num_devices=N_CORES)

    # inputs: [128, ic, d, n] with (p, i, d, n) = x.T[d*128+p, i*512+n]
    qT = nc.dram_tensor("qT", [128, N_ICS, ND, IC], BF, kind="ExternalInput").ap()
    kT = nc.dram_tensor("kT", [128, N_ICS, ND, IC], BF, kind="ExternalInput").ap()
    vT = nc.dram_tensor("vT", [128, N_ICS, ND, IC], BF, kind="ExternalInput").ap()
    wqkv = nc.dram_tensor("wqkv", [128, 3, ND, E], BF,
                          kind="ExternalInput").ap()
    wo = nc.dram_tensor("wo", [E, DIM], BF, kind="ExternalInput").ap()
    mb = nc.dram_tensor("mb", [JT, n_mb, IC], BF, kind="ExternalInput").ap()
    out = nc.dram_tensor("out", [N, DIM], BF, kind="ExternalOutput").ap()

    with tile.TileContext(nc) as tc:
        with (
            tc.tile_pool(name="consts", bufs=1) as consts,
            tc.tile_pool(name="load", bufs=6) as loadp,
            tc.tile_pool(name="big", bufs=1) as bigp,
            tc.tile_pool(name="vt", bufs=3) as vtp,
            tc.tile_pool(name="pt", bufs=5) as ptp,
            tc.tile_pool(name="sm", bufs=10) as smp,
            tc.tile_pool(name="ot", bufs=10) as otp,
            tc.tile_pool(name="mmps", bufs=2, space="PSUM") as mmps,
            tc.tile_pool(name="sps", bufs=2, space="PSUM") as sps,
            tc.tile_pool(name="accps", bufs=2, space="PSUM") as accps,
        ):
            # ---- PE warmup on a memset tile: no DMA dependency ----
            zt = consts.tile([128, 512], BF, name="zt")
            nc.vector.memset(zt[:], 0.0)
            for wi in range(14):
                wps = mmps.tile([128, 256], F32, tag="mm", name=f"warm{wi}",
                                padded_shape=[128, 512])
                nc.tensor.matmul(wps[:], zt[:, 0:128], zt[:, 0:256],
                                 start=True, stop=True)

            # ---- constant loads (gpsimd ring) ----
            wqkv_sb = consts.tile([128, 3, ND, E], BF, name="wqkv_sb")
            nc.gpsimd.dma_start(wqkv_sb[:], wqkv[:])
            w_sb = {"wq": wqkv_sb[:, 0], "wk": wqkv_sb[:, 1],
                    "wv": wqkv_sb[:, 2]}
            ident = consts.tile([128, 128], BF)
            masks.make_identity(nc, ident[:])
            ones64 = consts.tile([1, DH], BF, name="ones64")
            nc.vector.memset(ones64[:], 1.0)

            # qhT/khT [e=128, n]: head0 rows 0:64, head1 rows 64:128
            qhT = bigp.tile([128, N], BF, tag="qhT")
            khT = bigp.tile([128, N], BF, tag="khT")
            # vh1 [j=128, jt, 130]: per key-tile [vh_h0 | 1 | vh_h1 | 1]
            vh1 = bigp.tile([JT, N_JTS, 130], BF, tag="vh1")
            nc.vector.memset(vh1[:, :, 64:65], 1.0)
            nc.vector.memset(vh1[:, :, 129:130], 1.0)
            normT = bigp.tile([128, N], BF, tag="normT")

            in_sb = {}

            def emit_load1(name, srcap, i):
                t = loadp.tile([128, ND, IC], BF, tag="ld",
                               name=f"in_{name}{i}")
                if i < 2:
                    nd2 = ND // 2
                    nc.sync.dma_start(t[:, 0:nd2, :], srcap[:, i, 0:nd2, :])
                    nc.sync.dma_start(t[:, nd2:, :], srcap[:, i, nd2:, :])
                else:
                    nc.sync.dma_start(t[:], srcap[:, i])
                in_sb[(name, i)] = t

            def emit_load(i):
                for name, srcap in (("q", qT), ("k", kT), ("v", vT)):
                    emit_load1(name, srcap, i)

            # ---- emission quanta ----

            def proj_quanta(i):
                quanta = []

                def mk_qk(name, w, dst):
                    def emit():
                        src = in_sb.pop((name[1], i))
                        ps = mmps.tile([128, IC], F32, tag="mm",
                                       name=f"ps_{name}{i}")
                        for d in range(ND):
                            nc.tensor.matmul(ps[:], w[:, d, :], src[:, d, :],
                                             start=(d == 0), stop=(d == ND - 1))
                        nc.vector.tensor_copy(dst[:, i * IC:(i + 1) * IC], ps[:])
                    return emit

                quanta.append(mk_qk("wq", w_sb["wq"], qhT))
                quanta.append(mk_qk("wk", w_sb["wk"], khT))

                vt_sb = vtp.tile([128, IC], BF, tag="vt", name=f"vt{i}")

                def emit_v():
                    src = in_sb.pop(("v", i))
                    ps = mmps.tile([128, IC], F32, tag="mm", name=f"ps_wv{i}")
                    for d in range(ND):
                        nc.tensor.matmul(ps[:], w_sb["wv"][:, d, :], src[:, d, :],
                                         start=(d == 0), stop=(d == ND - 1))
                    nc.vector.tensor_copy(vt_sb[:], ps[:])
                quanta.append(emit_v)

                def mk_tr(j4):
                    def emit():
                        jt = i * (IC // JT) + j4
                        tp = mmps.tile([128, JT], BF, tag="mm", name=f"tp{jt}")
                        nc.tensor.matmul(tp[:], vt_sb[:, j4 * JT:(j4 + 1) * JT],
                                         ident[:], is_transpose=True)
                        nc.vector.tensor_copy(vh1[:, jt, 0:DH], tp[:, 0:DH])
                        nc.vector.tensor_copy(vh1[:, jt, 65:65 + DH],
                                              tp[:, DH:2 * DH])
                    return emit

                for j4 in range(IC // JT):
                    quanta.append(mk_tr(j4))
                return quanta

            ot_cur = {}

            def emit_outproj(it, oc):
                po = mmps.tile([128, 512], F32, tag="mm",
                               name=f"po{it}_{oc}")
                nc.tensor.matmul(po[:],
                                 normT[:, it * JT:(it + 1) * JT],
                                 wo_sb[:, oc * 512:(oc + 1) * 512],
                                 start=True, stop=True)
                if oc == 0:
                    ot_cur[it] = otp.tile([128, 2, 512], BF, tag="ot",
                                          name=f"ot{it}")
                ot = ot_cur[it]
                if (it + oc) % 2 == 0:
                    nc.scalar.copy(ot[:, oc, :], po[:])
                else:
                    nc.vector.tensor_copy(ot[:, oc, :], po[:])
                if oc == 1:
                    nc.sync.dma_start(out[it * JT:(it + 1) * JT, :],
                                      ot_cur.pop(it)[:])

            def attn_emitters(ic):
                """Per-k-tile emitters + finalizer. Scores/exp/mask/PV all
                clipped to the block's q128-aligned band. accs is opened by
                a full-span zeroing matmul so every PV matmul is a pure
                accumulate — per-sub start flags would clobber sibling
                chains sharing the PSUM bank."""
                lst, bclose = chunks[ic]  # bins + fin0 closing bin
                nt = len(lst)
                accs = [accps.tile([65, IC], F32, tag="acc",
                                   name=f"acc{ic}_{h}") for h in range(HPC)]
                state = {"pend": [], "mi": 0, "opened": False}

                def open_accs():
                    # deferred: the zero-openers WAR-wait on the previous
                    # chunk's finalize; emitting them after bin 1 keeps the
                    # in-order PE queue fed meanwhile
                    if not state["opened"]:
                        state["opened"] = True
                        for h in range(HPC):
                            nc.tensor.matmul(accs[h][:], zt[:, 0:65], zt[:],
                                             start=True, stop=False,
                                             skip_group_check=True)

                def emit_pv(pt, t):
                    open_accs()
                    for (jt, lo, hi, u, off) in lst[t]:
                        w = hi - lo
                        for h in range(HPC):
                            nc.tensor.matmul(
                                accs[h][:, lo:hi],
                                vh1[:, jt, 65 * h:65 * h + 65],
                                pt[:, h, off:off + w],
                                start=False, stop=(t == nt - 1),
                                skip_group_check=True)

                def mk_tile(t):
                    def emit():
                        bn = lst[t]
                        bw = bn[-1][4] + bn[-1][2] - bn[-1][1]
                        sp = sps.tile([128, HPC, IC], F32, tag="s",
                                      name=f"s{ic}_{t}")
                        for (jt, lo, hi, u, off) in bn:
                            w = hi - lo
                            for h in range(HPC):
                                nc.tensor.matmul(
                                    sp[:, h, off:off + w],
                                    khT[h * DH:(h + 1) * DH,
                                        jt * JT:(jt + 1) * JT],
                                    qhT[h * DH:(h + 1) * DH,
                                        ic * IC + lo:ic * IC + hi],
                                    start=True, stop=True)
                        pt = ptp.tile([JT, HPC, IC], BF, tag="pt",
                                      name=f"pt{ic}_{t}")
                        nc.scalar.activation(
                            pt[:, :, 0:bw], sp[:, :, 0:bw],
                            mybir.ActivationFunctionType.Exp, scale=SCALE)
                        for (jt, lo, hi, u, off) in bn:
                            if u < 0:
                                continue
                            w = hi - lo
                            for h in range(HPC):
                                nc.vector.tensor_mul(
                                    pt[:, h, off:off + w],
                                    pt[:, h, off:off + w],
                                    mb_sb[:, u, lo:hi])
                        state["pend"].append((pt, t))
                    return emit

                def fin_half(half):
                    cs, ce = half * 256, (half + 1) * 256
                    tail = ic == N_ICS - 1
                    rcs = []
                    for h in range(HPC):
                        rs = smp.tile([1, 256], F32, tag="rs",
                                      name=f"rs{ic}_{half}_{h}")
                        nc.scalar.copy(rs[:], accs[h][64:65, cs:ce])
                        rc = smp.tile([1, 256], F32, tag="rc",
                                      name=f"rc{ic}_{half}_{h}")
                        nc.vector.reciprocal_approx_fast(rc[:], rs[:])
                        rcs.append(rc)
                    for h in range(HPC):
                        bc = smp.tile([DH, 256], F32, tag="bc",
                                      name=f"bc{ic}_{half}_{h}")
                        nc.gpsimd.partition_broadcast(bc[:], rcs[h][:])
                        nc.vector.tensor_mul(
                            normT[h * DH:(h + 1) * DH,
                                  ic * IC + cs:ic * IC + ce],
                            accs[h][0:DH, cs:ce], bc[:])

                def emit_half(half, with_outproj, fin=True):
                    if fin:
                        fin_half(half)
                    if with_outproj:
                        for j4 in range(2):
                            it = ic * NSUB + half * 2 + j4
                            for oc in range(2):
                                emit_outproj(it, oc)

                def drain(n):
                    while len(state["pend"]) > n:
                        emit_pv(*state["pend"].pop(0))

                return ([mk_tile(t) for t in range(len(lst))], drain,
                        emit_half, open_accs)

            # ---- interleaved schedule ----
            emit_load1("q", qT, 0)
            emit_load1("k", kT, 0)
            mb_sb = consts.tile([JT, n_mb, IC], BF, name="mb_sb")
            n0 = min(4, n_mb)
            nc.gpsimd.dma_start(mb_sb[:, 0:n0], mb[:, 0:n0])
            emit_load1("v", vT, 0)
            if n_mb > n0:
                nc.gpsimd.dma_start(mb_sb[:, n0:], mb[:, n0:])
            emit_load1("q", qT, 1)
            wo_sb = consts.tile([E, DIM], BF, name="wo_sb")
            nc.gpsimd.dma_start(wo_sb[:], wo[:])
            emit_load1("k", kT, 1)
            emit_load1("v", vT, 1)
            pq0 = proj_quanta(0)
            for q in pq0[:2]:       # q/k chains: unblock chunk-0 scores
                q()
            pq0_rest = pq0[2:]      # v chain + transposes follow bin 0

            for i in range(N_ICS):
                last = i == N_ICS - 1
                tiles_i, drain, emit_half, open_accs = attn_emitters(i)
                other = []
                if i == 0:
                    other.extend(pq0_rest)
                if i + 2 < N_ICS:
                    other.append(lambda i=i: emit_load(i + 2))
                if i + 1 < N_ICS:
                    other.extend(proj_quanta(i + 1))
                if i > 0:
                    # previous chunk's outproj, spread through this chunk
                    c = i - 1
                    for it in range(c * NSUB, (c + 1) * NSUB):
                        for oc in range(2):
                            other.append(
                                lambda it=it, oc=oc: emit_outproj(it, oc))
                npr = len(tiles_i)
                fin0_at = chunks[i][1] + 3  # PV(bclose) drains at t+3
                k = 0
                for t, p in enumerate(tiles_i):
                    # PE queue order per tile: oldest pending PV, then
                    # filler quanta (cover the exp(t-1) latency), then
                    # this tile's scores (which WAR-wait on exp(t-1)).
                    # Chunk 0's fillers all stall on in-flight DMAs and
                    # would block the in-order PE queue: bins go first.
                    drain(2)
                    want = 0 if i == 0 else (t * len(other)) // npr
                    while k < want:
                        other[k]()
                        k += 1
                    p()
                    if t == 1:
                        open_accs()
                    if t == fin0_at and t <= npr - 1:
                        emit_half(0, with_outproj=last)
                while k < len(other):
                    other[k]()
                    k += 1
                if fin0_at > npr - 1:
                    drain(npr - 1 - chunks[i][1])
                    # fin chain only; the final PVs below fill its latency
                    emit_half(0, with_outproj=False)
                    drain(0)
                    emit_half(0, with_outproj=last, fin=False)
                else:
                    drain(0)
                emit_half(1, with_outproj=last)

    nc.compile()
    return nc


def _rearr(xT):
    # [DIM, N] -> [128, N_ICS, ND, IC] bf16: (p, i, d, n) = xT[d*128+p, i*512+n]
    return np.ascontiguousarray(
        xT.reshape(ND, 128, N_ICS, IC).transpose(1, 2, 0, 3)).astype(BF_NP)


def _rearr_w(w):
    # [DIM, E] -> [128, ND, E]
    return np.ascontiguousarray(
        w.reshape(ND, 128, E)).transpose(1, 0, 2).astype(BF_NP)


_CACHE = {}


def kernel(q, k, v, Wq, Wk, Wv, Wo, bo, mask_block, _trace=False):
    q = np.asarray(q); k = np.asarray(k); v = np.asarray(v)
    Wq = np.asarray(Wq, np.float32); Wk = np.asarray(Wk, np.float32)
    Wv = np.asarray(Wv, np.float32); Wo = np.asarray(Wo, np.float32)
    bo = np.asarray(bo, np.float32)
    mask = np.asarray(mask_block)
    b, n, d = q.shape
    assert (b, n, d) == (1, N_FULL, DIM)

    chunks, mbt, pv_segs = _schedule(mask)
    n_mb = max(1, mbt.shape[1])

    key = (repr(chunks), n_mb)
    if key not in _CACHE:
        _CACHE[key] = _build(chunks, pv_segs, n_mb)
    nc = _CACHE[key]

    qTb = _rearr(q[0, :N].T.astype(np.float32))
    kTb = _rearr(k[0, :N].T.astype(np.float32))
    vTb = _rearr(v[0, :N].T.astype(np.float32))
    mbb = mbt.astype(BF_NP)

    in_maps = []
    for c in range(N_CORES):
        sl = slice(c * E, (c + 1) * E)
        wqkv = np.stack([_rearr_w(np.ascontiguousarray(W[:, sl]))
                         for W in (Wq, Wk, Wv)], axis=1)
        in_maps.append({
            "qT": qTb, "kT": kTb, "vT": vTb,
            "wqkv": np.ascontiguousarray(wqkv),
            "wo": np.ascontiguousarray(Wo[sl, :]).astype(BF_NP),
            "mb": mbb,
        })

    res = run_bass_kernel_spmd(
        nc, in_maps, core_ids=list(range(N_CORES)),
        trace=_trace, trace_cores=list(range(N_CORES)) if _trace else None)

    acc = res.results[0]["out"].astype(np.float32)
    for c in range(1, N_CORES):
        acc = acc + res.results[c]["out"].astype(np.float32)
    outf = np.empty((1, N_FULL, DIM), np.float32)
    outf[0, :N] = acc + bo

    am = np.where(mask.all(axis=1))[0]
    if am.size:
        vmean = v[0].astype(np.float32).mean(axis=0)
        row = (vmean @ Wv) @ Wo + bo
        outf[0, am] = row
    if _trace:
        kernel._last_exec_ns = res.exec_time_ns
        kernel._last_res = res
    return outf
